# revision 23
# baseline (speedup 1.0000x reference)
"""Trainium2 Bass kernel for nn_Backbone1_62947040690721.

Data-parallel over the fused B*NV block axis: 336 independent per-series
problems, 42 per NeuronCore across 8 cores. All weights replicated.

Layouts (per core, 42 blocks):
  - row layout: (block,patch) rows on partitions (128 rows = 2 blocks/tile)
  - T layout:   features on partitions, l = concat of blocks on the free axis,
                padded to 67 cols/block (3 zero "gap" cols + 64 data cols).
                The gaps give causal-conv zero padding and selective-scan
                state resets between blocks for free.
The selective scan runs as hardware tensor_tensor_scan instructions, one per
(state dim d, channel chunk), scanning 7 blocks' timelines per call; scan
chunks are block-aligned so every chunk self-resets at its leading gap cols.
"""

import sys

sys.path.insert(0, "/opt/trn_rl_repo")

import numpy as np

import concourse.bass as bass
import concourse.mybir as mybir
import concourse.tile as tile
from concourse import bacc
from concourse.bass_utils import run_bass_kernel_spmd

F32 = mybir.dt.float32
F32R = mybir.dt.float32r
AF = mybir.ActivationFunctionType
OP = mybir.AluOpType
AX = mybir.AxisListType

# model dims
B, T, NV = 16, 512, 21
PS, STRIDE, PRED = 16, 8, 96
DM, DS, DC = 128, 16, 4
DIN = 2 * DM          # 256
DTR = 8
S_EA = 512
PN = (T - PS) // STRIDE + 1 + 1  # 64
EPS = 1e-5

NCORES = 8
NBLK = B * NV          # 336
RPC = NBLK // NCORES   # 42 blocks per core
NROW = RPC * PN        # 2688 compact rows per core
NRT = NROW // 128      # 21 row tiles
GAP = 3                # zero-pad cols before each block
LP = PN + GAP          # 67 padded cols per block
LT = RPC * LP          # 2814 padded timeline length
SCB = 7                # blocks per scan chunk
SCW = SCB * LP         # 469 scan chunk width (>=256 keeps fp32r full rate)
NSC = RPC // SCB       # 6 scan chunks
POISON = 1.0e30

DEBUG = False          # set True (before first kernel build) for stage taps

_cache = {}


def _r(x):
    return np.ascontiguousarray(np.asarray(x, dtype=np.float32))


def prep_inputs(inputs):
    """Full inputs -> per-core input maps (pure data movement on host)."""
    x = _r(inputs["x"])
    xbn = np.ascontiguousarray(x.transpose(0, 2, 1).reshape(NBLK, T))
    xp = np.concatenate([xbn, np.repeat(xbn[:, -1:], STRIDE, axis=1)], axis=1)
    idx = np.arange(PN)[:, None] * STRIDE + np.arange(PS)[None, :]
    pat = xp[:, idx]                                     # (336, 64, 16)
    patT = np.ascontiguousarray(pat.transpose(2, 0, 1))  # (16, 336, 64)
    wv = np.tile(_r(inputs["revin_w"]), B).reshape(NBLK, 1)
    bv = np.tile(_r(inputs["revin_b"]), B).reshape(NBLK, 1)

    mlp2_wT = _r(inputs["mlp2_w"]).T       # (8192, 192)
    w2s = np.ascontiguousarray(
        mlp2_wT.reshape(PN, DM, 2 * PRED).transpose(1, 0, 2)
    )  # (128, 64, 192)

    shared = {
        "mlp1_wT": _r(inputs["mlp1_w"]).T.copy(),          # (16,128)
        "mlp1_b_row": _r(inputs["mlp1_b"]).reshape(1, DM),
        "mk_wT": _r(inputs["mk_w"]).T.copy(),              # (128,512)
        "mv_wT": _r(inputs["mv_w"]).T.copy(),              # (512,128)
        "ln_w_row": _r(inputs["ln_w"]).reshape(1, DM),
        "ln_b_row": _r(inputs["ln_b"]).reshape(1, DM),
        "in_proj_wT": _r(inputs["in_proj_w"]).T.copy(),    # (128,512)
        "conv_w2": _r(inputs["conv_w"])[:, 0, :].copy(),   # (256,4)
        "conv_b_col": _r(inputs["conv_b"]).reshape(DIN, 1),
        "x_proj_wT": _r(inputs["x_proj_w"]).T.copy(),      # (256,40)
        "dt_proj_wT": _r(inputs["dt_proj_w"]).T.copy(),    # (8,256)
        "dt_proj_b_col": _r(inputs["dt_proj_b"]).reshape(DIN, 1),
        "A_log_in": _r(inputs["A_log"]),                   # (256,16)
        "D_col": _r(inputs["D_ssm"]).reshape(DIN, 1),
        "out_proj_wT": _r(inputs["out_proj_w"]).T.copy(),  # (256,128)
        "w2s": w2s,                                        # (128,64,192)
        "mlp2_b_col": _r(inputs["mlp2_b"]).reshape(2 * PRED, 1),
        "mlp3_wT": _r(inputs["mlp3_w"]).T.copy(),          # (192,96)
        "mlp3_b_row": _r(inputs["mlp3_b"]).reshape(1, PRED),
    }
    in_maps = []
    for c in range(NCORES):
        lo, hi = c * RPC, (c + 1) * RPC
        m = dict(shared)
        m["xrow"] = np.ascontiguousarray(xbn[lo:hi])                  # (42,512)
        m["patT"] = np.ascontiguousarray(patT[:, lo:hi, :]).reshape(PS, NROW)
        m["wv"] = np.ascontiguousarray(wv[lo:hi])
        m["bv"] = np.ascontiguousarray(bv[lo:hi])
        in_maps.append(m)
    return in_maps


def assemble(results):
    outs = np.concatenate([r["out"] for r in results], axis=0)  # (336, 96)
    out = outs.reshape(B, NV, PRED).transpose(0, 2, 1)
    return np.ascontiguousarray(out.astype(np.float32))


# ---------------------------------------------------------------------------
# program builder
# ---------------------------------------------------------------------------

def _decl_inputs(nc):
    d = {}
    spec = {
        "xrow": (RPC, T), "patT": (PS, NROW), "wv": (RPC, 1), "bv": (RPC, 1),
        "mlp1_wT": (PS, DM), "mlp1_b_row": (1, DM),
        "mk_wT": (DM, S_EA), "mv_wT": (S_EA, DM),
        "ln_w_row": (1, DM), "ln_b_row": (1, DM),
        "in_proj_wT": (DM, 2 * DIN),
        "conv_w2": (DIN, DC), "conv_b_col": (DIN, 1),
        "x_proj_wT": (DIN, DTR + 2 * DS),
        "dt_proj_wT": (DTR, DIN), "dt_proj_b_col": (DIN, 1),
        "A_log_in": (DIN, DS), "D_col": (DIN, 1),
        "out_proj_wT": (DIN, DM),
        "w2s": (DM, PN, 2 * PRED), "mlp2_b_col": (2 * PRED, 1),
        "mlp3_wT": (2 * PRED, PRED), "mlp3_b_row": (1, PRED),
    }
    for name, shape in spec.items():
        d[name] = nc.dram_tensor(name, list(shape), F32, kind="ExternalInput").ap()
    return d


def build_program():
    if "nc" in _cache:
        return _cache["nc"]
    nc = bacc.Bacc("TRN2", target_bir_lowering=False, debug=False,
                   num_devices=NCORES)
    IN = _decl_inputs(nc)
    out_d = nc.dram_tensor("out", [RPC, PRED], F32, kind="ExternalOutput").ap()

    dbg = {}
    if DEBUG:
        for name, shape in [
            ("d_hT", (DM, NROW)), ("d_hbT", (DM, NROW)),
            ("d_xc2T", (DIN, LT)), ("d_deltaT", (DIN, LT)),
            ("d_duT", (DIN, LT)), ("d_y2T", (DIN, LT)),
            ("d_moT", (DM, LT)), ("d_dblT", (DTR + 2 * DS, LT)),
            ("d_dblB", (DS, LT)), ("d_dblC", (DS, LT)),
            ("d_stats", (RPC, 6)),
        ]:
            dty = mybir.dt.bfloat16 if name in ("d_dblB", "d_dblC") else F32
            dbg[name] = nc.dram_tensor(name, list(shape), dty,
                                       kind="ExternalOutput").ap()

    from contextlib import ExitStack
    from concourse.masks import make_identity

    with tile.TileContext(nc) as tc, ExitStack() as ctx:
        P = lambda **kw: ctx.enter_context(tc.tile_pool(**kw))
        wpool = P(name="weights", bufs=1)
        cpool = P(name="consts", bufs=1)
        spool = P(name="statp", bufs=1)
        big = P(name="bigact", bufs=1)
        work = P(name="work", bufs=2)
        work2 = P(name="work2", bufs=2)
        scanp = P(name="scanp", bufs=2)
        w2pool = P(name="w2p", bufs=2)
        # PSUM: mm(2 banks) + bc(4 banks) + y(2 banks) = 8 banks
        ps_mm = P(name="ps_mm", bufs=2, space="PSUM")
        ps_bc = P(name="ps_bc", bufs=2, space="PSUM")
        ps_y = P(name="ps_y", bufs=1, space="PSUM")

        dt = F32

        def dma(dst, src):
            nc.sync.dma_start(out=dst, in_=src)

        def mm_tile(shape, tag="mm"):
            return ps_mm.tile(list(shape), dt, tag=tag, name=tag)

        # ---- constants / weights to SBUF ----
        ident = cpool.tile([128, 128], dt)
        make_identity(nc, ident[:])
        identb = cpool.tile([128, 128], mybir.dt.bfloat16)
        make_identity(nc, identb[:])
        ones1 = cpool.tile([1, 128], dt)
        nc.vector.memset(ones1[:], 1.0)
        ones16 = cpool.tile([PS, 1], dt)
        nc.vector.memset(ones16[:], 1.0)
        epsc = cpool.tile([128, 1], dt)
        nc.vector.memset(epsc[:], EPS)

        w = {}
        for name, shape in [
            ("mlp1_wT", (PS, DM)), ("mk_wT", (DM, S_EA)),
            ("in_proj_wT", (DM, 2 * DIN)), ("dt_proj_wT", (DTR, DIN)),
        ]:
            tl = wpool.tile(list(shape), dt, tag=name)
            dma(tl[:], IN[name])
            w[name] = tl
        # channel-chunked weights (DIN=256 or 192 rows -> per-128 tiles)
        for name, shape in [
            ("conv_w2", (DIN, DC)), ("conv_b_col", (DIN, 1)),
            ("x_proj_wT", (DIN, DTR + 2 * DS)), ("dt_proj_b_col", (DIN, 1)),
            ("D_col", (DIN, 1)), ("out_proj_wT", (DIN, DM)),
            ("mlp2_b_col", (2 * PRED, 1)), ("mlp3_wT", (2 * PRED, PRED)),
        ]:
            rows = shape[0]
            parts = []
            for cc in range((rows + 127) // 128):
                r0 = cc * 128
                r1 = min(rows, r0 + 128)
                tl = wpool.tile([r1 - r0, shape[1]], dt, tag=f"{name}{cc}")
                dma(tl[:], IN[name][r0:r1, :])
                parts.append(tl)
            w[name] = parts

        # mv_aug: mv_wT chunks with an appended ones column (for softmax sums)
        mv_aug = wpool.tile([128, 4 * (DM + 1)], dt)
        for sc in range(4):
            dma(mv_aug[:, sc * 129:sc * 129 + DM],
                IN["mv_wT"][sc * 128:(sc + 1) * 128, :])
            nc.vector.memset(mv_aug[:, sc * 129 + DM:(sc + 1) * 129], 1.0)

        # A = -exp(A_log), (128,16) per channel chunk
        A_sb = []
        for cc in range(2):
            raw = work.tile([128, DS], dt, tag="araw")
            dma(raw[:], IN["A_log_in"][cc * 128:(cc + 1) * 128, :])
            ex = work.tile([128, DS], dt, tag="aexp")
            nc.scalar.activation(ex[:], raw[:], AF.Exp)
            neg = wpool.tile([128, DS], dt, tag=f"A_{cc}")
            nc.vector.tensor_scalar_mul(neg[:], ex[:], -1.0)
            A_sb.append(neg)

        # broadcast a (1,width) DRAM row -> (128,width) SBUF tile
        def bcast_row(dram_row, width, tag):
            row = work.tile([1, width], dt, tag="brow")
            dma(row[:], dram_row)
            ps = mm_tile([128, width])
            nc.tensor.matmul(ps[:], ones1[:], row[:], start=True, stop=True)
            sb = cpool.tile([128, width], dt, tag=tag)
            nc.scalar.copy(sb[:], ps[:])
            return sb

        b1_bc = bcast_row(IN["mlp1_b_row"], DM, "b1bc")
        lnw_bc = bcast_row(IN["ln_w_row"], DM, "lnwbc")
        lnb_bc = bcast_row(IN["ln_b_row"], DM, "lnbbc")

        # w1sum[dm] = sum_ps mlp1_wT -> broadcast tile
        ps_w1 = mm_tile([DM, 1])
        nc.tensor.matmul(ps_w1[:], w["mlp1_wT"][:], ones16[:], start=True, stop=True)
        w1s_col = work.tile([DM, 1], dt, tag="w1c")
        nc.vector.tensor_copy(w1s_col[:], ps_w1[:])
        ps_w1r = mm_tile([1, DM])
        nc.tensor.transpose(ps_w1r[:], w1s_col[:], ident[:])
        w1s_row = work.tile([1, DM], dt, tag="w1r")
        nc.vector.tensor_copy(w1s_row[:], ps_w1r[:])
        ps_w1b = mm_tile([128, DM])
        nc.tensor.matmul(ps_w1b[:], ones1[:], w1s_row[:], start=True, stop=True)
        w1s_bc = cpool.tile([128, DM], dt)
        nc.scalar.copy(w1s_bc[:], ps_w1b[:])

        # ---- stage A: RevIN stats ----
        xr = big.tile([RPC, T], dt, tag="xrow")
        dma(xr[:], IN["xrow"])
        wv = spool.tile([RPC, 1], dt)
        dma(wv[:], IN["wv"])
        bv = spool.tile([RPC, 1], dt)
        dma(bv[:], IN["bv"])

        sumx = spool.tile([RPC, 1], dt)
        nc.vector.reduce_sum(sumx[:], xr[:], axis=AX.X)
        mean = spool.tile([RPC, 1], dt)
        nc.vector.tensor_scalar_mul(mean[:], sumx[:], 1.0 / T)
        sq = work.tile([RPC, T], dt, tag="sq", bufs=1)
        sumx2 = spool.tile([RPC, 1], dt)
        nc.scalar.activation(sq[:], xr[:], AF.Square, accum_out=sumx2[:])
        ex2 = spool.tile([RPC, 1], dt)
        nc.vector.tensor_scalar_mul(ex2[:], sumx2[:], 1.0 / T)
        msq = spool.tile([RPC, 1], dt)
        nc.vector.tensor_mul(msq[:], mean[:], mean[:])
        var = spool.tile([RPC, 1], dt)
        nc.vector.tensor_sub(var[:], ex2[:], msq[:])
        varp = spool.tile([RPC, 1], dt)
        nc.vector.tensor_scalar_add(varp[:], var[:], EPS)
        std = spool.tile([RPC, 1], dt)
        nc.scalar.activation(std[:], varp[:], AF.Sqrt)
        istd = spool.tile([RPC, 1], dt)
        nc.vector.reciprocal(istd[:], std[:])

        s_n = spool.tile([RPC, 1], dt)
        nc.vector.tensor_mul(s_n[:], wv[:], istd[:])
        o_n0 = spool.tile([RPC, 1], dt)
        nc.vector.scalar_tensor_tensor(o_n0[:], mean[:], -1.0, s_n[:],
                                       op0=OP.mult, op1=OP.mult)
        o_n = spool.tile([RPC, 1], dt)
        nc.vector.tensor_add(o_n[:], o_n0[:], bv[:])

        wq = spool.tile([RPC, 1], dt)
        nc.vector.tensor_scalar_add(wq[:], wv[:], EPS * EPS)
        rw = spool.tile([RPC, 1], dt)
        nc.vector.reciprocal(rw[:], wq[:])
        t_den = spool.tile([RPC, 1], dt)
        nc.vector.tensor_mul(t_den[:], std[:], rw[:])
        u_den0 = spool.tile([RPC, 1], dt)
        nc.vector.scalar_tensor_tensor(u_den0[:], bv[:], -1.0, t_den[:],
                                       op0=OP.mult, op1=OP.mult)
        u_den = spool.tile([RPC, 1], dt)
        nc.vector.tensor_add(u_den[:], u_den0[:], mean[:])

        svec = spool.tile([RPC, 2], dt)
        nc.vector.tensor_copy(svec[:, 0:1], s_n[:])
        nc.vector.tensor_copy(svec[:, 1:2], o_n[:])
        if DEBUG:
            stats = spool.tile([RPC, 6], dt)
            for i, tl in enumerate([mean, std, s_n, o_n, t_den, u_den]):
                nc.vector.tensor_copy(stats[:, i:i + 1], tl[:])
            dma(dbg["d_stats"], stats[:])

        # ---- stage B: mlp1 + external attention + LN + gelu + residual ----
        # structured as function-grouped passes to avoid ACT table thrash
        hT = big.tile([DM, NROW], dt, tag="hT")
        hbT = big.tile([DM, NROW], dt, tag="hbT")
        hrow_all = big.tile([128, NRT, DM], dt, tag="sluz0")
        an_all = big.tile([128, NRT, DM], dt, tag="sluz1")
        exp_all = [big.tile([128, NROW], dt, tag=tg, name=f"exp_all{i}")
                   for i, tg in enumerate(["xcT0", "xcT1", "xc2T0", "xc2T1"])]

        # B1: mlp1 + revin fold + transpose -> hT, hrow_all
        for rt in range(NRT):
            cs = rt * 128
            so_row = work.tile([128, 2], dt, tag="so_row")
            dma(so_row[:],
                svec[rt * 2:rt * 2 + 2, :].unsqueeze(1).broadcast_to((2, PN, 2)))
            patt = work.tile([PS, 128], dt, tag="patt")
            dma(patt[:], IN["patT"][:, cs:cs + 128])
            ps_h = mm_tile([128, DM])
            nc.tensor.matmul(ps_h[:], patt[:], w["mlp1_wT"][:],
                             start=True, stop=True)
            t1 = work.tile([128, DM], dt, tag="t1")
            nc.vector.scalar_tensor_tensor(t1[:], w1s_bc[:], so_row[:, 1:2],
                                           b1_bc[:], op0=OP.mult, op1=OP.add)
            nc.vector.scalar_tensor_tensor(hrow_all[:, rt, :], ps_h[:],
                                           so_row[:, 0:1], t1[:],
                                           op0=OP.mult, op1=OP.add)
            ps_tr = mm_tile([DM, 128])
            nc.tensor.transpose(ps_tr[:], hrow_all[:, rt, :], ident[:])
            nc.scalar.copy(hT[:, cs:cs + 128], ps_tr[:])

        # B2: logits + exp (exp table)
        for rt in range(NRT):
            cs = rt * 128
            for sc in range(4):
                ps_l = mm_tile([128, 128])
                nc.tensor.matmul(ps_l[:], w["mk_wT"][:, sc * 128:(sc + 1) * 128],
                                 hT[:, cs:cs + 128], start=True, stop=True)
                nc.scalar.activation(exp_all[sc][:, cs:cs + 128], ps_l[:], AF.Exp)

        # B3: attnv (+sum column) + normalize
        for rt in range(NRT):
            cs = rt * 128
            ps_at = ps_y.tile([128, DM + 1], dt, tag="ps_y0", name="ps_at")
            for sc in range(4):
                nc.tensor.matmul(ps_at[:], exp_all[sc][:, cs:cs + 128],
                                 mv_aug[:, sc * 129:(sc + 1) * 129],
                                 start=(sc == 0), stop=(sc == 3))
            rec = work.tile([128, 1], dt, tag="rec")
            nc.vector.reciprocal(rec[:], ps_at[:, DM:DM + 1])
            nc.vector.tensor_scalar_mul(an_all[:, rt, :], ps_at[:, 0:DM], rec[:])

        # B4: LayerNorm over DM (sqrt table)
        for rt in range(NRT):
            a_n = an_all[:, rt, :]
            sm = work.tile([128, 1], dt, tag="sm")
            nc.vector.reduce_sum(sm[:], a_n, axis=AX.X)
            mu = work.tile([128, 1], dt, tag="mu")
            nc.vector.tensor_scalar_mul(mu[:], sm[:], 1.0 / DM)
            sqs = work2.tile([128, DM], dt, tag="sqs")
            ssq = work.tile([128, 1], dt, tag="ssq")
            nc.scalar.activation(sqs[:], a_n, AF.Square, accum_out=ssq[:])
            ex2r = work.tile([128, 1], dt, tag="ex2r")
            nc.vector.tensor_scalar_mul(ex2r[:], ssq[:], 1.0 / DM)
            msqr = work.tile([128, 1], dt, tag="msqr")
            nc.vector.tensor_mul(msqr[:], mu[:], mu[:])
            varr = work.tile([128, 1], dt, tag="varr")
            nc.vector.tensor_sub(varr[:], ex2r[:], msqr[:])
            sdr = work.tile([128, 1], dt, tag="sdr")
            nc.scalar.activation(sdr[:], varr[:], AF.Sqrt, bias=epsc[:])
            rstd = work.tile([128, 1], dt, tag="rstd")
            nc.vector.reciprocal(rstd[:], sdr[:])
            m2 = work.tile([128, 1], dt, tag="m2")
            nc.vector.scalar_tensor_tensor(m2[:], mu[:], -1.0, rstd[:],
                                           op0=OP.mult, op1=OP.mult)
            q = work2.tile([128, DM], dt, tag="q")
            nc.vector.tensor_scalar(q[:], a_n, rstd[:], m2[:],
                                    op0=OP.mult, op1=OP.add)
            ln = work2.tile([128, DM], dt, tag="ln")
            nc.vector.tensor_mul(ln[:], q[:], lnw_bc[:])
            nc.vector.tensor_add(an_all[:, rt, :], ln[:], lnb_bc[:])

        # B5: gelu + residual + transpose -> hbT (gelu table)
        for rt in range(NRT):
            cs = rt * 128
            g = work2.tile([128, DM], dt, tag="g")
            nc.scalar.activation(g[:], an_all[:, rt, :], AF.Gelu)
            hb_row = work2.tile([128, DM], dt, tag="hb_row")
            nc.vector.tensor_add(hb_row[:], g[:], hrow_all[:, rt, :])
            ps_tb = mm_tile([DM, 128])
            nc.tensor.transpose(ps_tb[:], hb_row[:], ident[:])
            nc.scalar.copy(hbT[:, cs:cs + 128], ps_tb[:])

        if DEBUG:
            dma(dbg["d_hT"], hT[:])
            dma(dbg["d_hbT"], hbT[:])

        # ---- stage D: in_proj -> xcT (padded); z -> silu_z (padded) ----
        xcT = [big.tile([128, LT], dt, tag=f"xcT{cc}", name=f"xcT{cc}") for cc in range(2)]
        sluz = [big.tile([128, LT], dt, tag=f"sluz{cc}", name=f"sluz{cc}") for cc in range(2)]
        for cc in range(2):
            nc.vector.memset(xcT[cc][:], 0.0)
            nc.vector.memset(sluz[cc][:], 0.0)
        ccw = [(i * 512, min(512, NROW - i * 512))
               for i in range((NROW + 511) // 512)]
        for pc in range(4):
            cchunk, isx = (pc % 2), (pc < 2)
            for (c0, cw) in ccw:
                nblk_c = cw // PN
                ps_x = mm_tile([128, 512])
                nc.tensor.matmul(ps_x[:, :cw],
                                 w["in_proj_wT"][:, pc * 128:(pc + 1) * 128],
                                 hbT[:, c0:c0 + cw], start=True, stop=True)
                p0 = (c0 // PN) * LP
                dst = (xcT[cchunk] if isx else sluz[cchunk])
                dview = dst[:, p0:p0 + nblk_c * LP].rearrange(
                    "p (b l) -> p b l", b=nblk_c)[:, :, GAP:LP]
                sview = ps_x[:, :cw].rearrange("p (b l) -> p b l", b=nblk_c)
                if isx:
                    nc.scalar.copy(dview, sview)
                else:
                    nc.scalar.activation(dview, sview, AF.Silu)

        # ---- stage E: causal depthwise conv + silu (chunked, no in-place) ----
        xc2T = [big.tile([128, LT], dt, tag=f"xc2T{cc}", name=f"xc2T{cc}")
                for cc in range(2)]
        for cc in range(2):
            nc.vector.memset(xc2T[cc][:], 0.0)
            wsl = w["conv_w2"][cc]
            for si in range(NSC):
                c0 = si * SCW
                cw_ = SCW - GAP
                t1c = scanp.tile([128, cw_], dt, tag="a_t", name="cv1")
                nc.vector.tensor_scalar(t1c[:], xcT[cc][:, c0:c0 + cw_],
                                        wsl[:, 0:1], None, op0=OP.mult)
                t2c = scanp.tile([128, cw_], dt, tag="b_t", name="cv2")
                nc.vector.scalar_tensor_tensor(t2c[:],
                                               xcT[cc][:, c0 + 1:c0 + 1 + cw_],
                                               wsl[:, 1:2], t1c[:],
                                               op0=OP.mult, op1=OP.add)
                t3c = scanp.tile([128, cw_], dt, tag="a_t", name="t3c")
                nc.vector.scalar_tensor_tensor(t3c[:],
                                               xcT[cc][:, c0 + 2:c0 + 2 + cw_],
                                               wsl[:, 2:3], t2c[:],
                                               op0=OP.mult, op1=OP.add)
                t4c = scanp.tile([128, cw_], dt, tag="b_t", name="t4c")
                nc.vector.scalar_tensor_tensor(t4c[:],
                                               xcT[cc][:, c0 + 3:c0 + 3 + cw_],
                                               wsl[:, 3:4], t3c[:],
                                               op0=OP.mult, op1=OP.add)
                nc.scalar.activation(xc2T[cc][:, c0 + GAP:c0 + SCW], t4c[:],
                                     AF.Silu, bias=w["conv_b_col"][cc][:])
        if DEBUG:
            for cc in range(2):
                dma(dbg["d_xc2T"][cc * 128:(cc + 1) * 128, :], xc2T[cc][:])

        # ---- stage F: x_proj -> (dt,Bm,Cm); dt_proj -> delta; du ----
        # separate tiles so each starts at partition 0 (matmul base rule)
        dblD = big.tile([DTR, LT], dt, tag="hT")  # reuse hT slot (dead)
        dblB_t = big.tile([DS, LT], mybir.dt.bfloat16, tag="dblB")
        dblC_t = big.tile([DS, LT], mybir.dt.bfloat16, tag="dblC")
        dblB = dblB_t[:]
        dblC = dblC_t[:]
        for si in range(NSC):
            c0 = si * SCW
            for (lo, hi, dst) in [(0, DTR, dblD[:]), (DTR, DTR + DS, dblB),
                                  (DTR + DS, DTR + 2 * DS, dblC)]:
                ps_d = mm_tile([hi - lo, SCW])
                for cc in range(2):
                    nc.tensor.matmul(ps_d[:],
                                     w["x_proj_wT"][cc][:, lo:hi],
                                     xc2T[cc][:, c0:c0 + SCW],
                                     start=(cc == 0), stop=(cc == 1))
                nc.scalar.copy(dst[:, c0:c0 + SCW], ps_d[:])
        if DEBUG:
            dma(dbg["d_dblT"][0:DTR, :], dblD[:])
            dma(dbg["d_dblB"], dblB)
            dma(dbg["d_dblC"], dblC)

        deltaT = [big.tile([128, LT], dt, tag=f"xcT{cc}", name=f"deltaT{cc}") for cc in range(2)]
        duT = [big.tile([128, LT], dt, tag=t, name=f"duT_{t}") for t in ("convacc", "hbT")]
        for cc in range(2):
            for si in range(NSC):
                c0 = si * SCW
                ps_dt = mm_tile([128, SCW])
                nc.tensor.matmul(ps_dt[:],
                                 w["dt_proj_wT"][:, cc * 128:(cc + 1) * 128],
                                 dblD[:][:, c0:c0 + SCW], start=True, stop=True)
                e1 = scanp.tile([128, SCW], dt, tag="a_t", name="e1")
                nc.scalar.activation(e1[:], ps_dt[:], AF.Exp,
                                     bias=w["dt_proj_b_col"][cc][:])
                nc.scalar.activation(deltaT[cc][:, c0:c0 + SCW], e1[:],
                                     AF.Ln, bias=1.0)
            # du = delta * xc2 on data cols; gaps zero
            nc.vector.memset(duT[cc][:], 0.0)
            nc.vector.tensor_mul(
                duT[cc][:].rearrange("p (b l) -> p b l", b=RPC)[:, :, GAP:LP],
                deltaT[cc][:].rearrange("p (b l) -> p b l", b=RPC)[:, :, GAP:LP],
                xc2T[cc][:].rearrange("p (b l) -> p b l", b=RPC)[:, :, GAP:LP])
            # poison delta gaps so exp(A*delta)=0 there (scan state reset)
            nc.vector.memset(
                deltaT[cc][:].rearrange("p (b l) -> p b l", b=RPC)[:, :, 0:GAP],
                POISON)
        if DEBUG:
            for cc in range(2):
                dma(dbg["d_deltaT"][cc * 128:(cc + 1) * 128, :], deltaT[cc][:])
                dma(dbg["d_duT"][cc * 128:(cc + 1) * 128, :], duT[cc][:])

        # ---- stage G: selective scan ----
        # one-hot row-selection matrix: sel[i, d*128+m] = (i == d)
        sel = cpool.tile([DS, DS * 128], mybir.dt.bfloat16)
        nc.gpsimd.memset(sel[:], 0.0)
        nc.gpsimd.affine_select(out=sel[:], in_=sel[:],
                                compare_op=OP.not_equal, fill=1.0,
                                base=0, pattern=[[-1, DS], [0, 128]],
                                channel_multiplier=1)
        y2T = [big.tile([128, LT], dt, tag=f"y2T{cc}", name=f"y2T{cc}") for cc in range(2)]

        for si in range(NSC):
            c0 = si * SCW
            ps_ys = [ps_y.tile([128, SCW], dt, tag=f"ps_y{cc}", name=f"ps_ys{cc}")
                     for cc in range(2)]
            for d in range(DS):
                ps_bm = ps_bc.tile([128, SCW], dt, tag="ps_bm")
                nc.tensor.matmul(ps_bm[:],
                                 sel[:, d * 128:(d + 1) * 128],
                                 dblB[:, c0:c0 + SCW],
                                 start=True, stop=True)
                ps_cm = ps_bc.tile([128, SCW], dt, tag="ps_cm")
                nc.tensor.matmul(ps_cm[:],
                                 sel[:, d * 128:(d + 1) * 128],
                                 dblC[:, c0:c0 + SCW],
                                 start=True, stop=True)
                for cc in range(2):
                    a_t = scanp.tile([128, SCW], dt, tag="a_t")
                    nc.scalar.activation(a_t[:], deltaT[cc][:, c0:c0 + SCW],
                                         AF.Exp, scale=A_sb[cc][:, d:d + 1])
                    b_t = scanp.tile([128, SCW], dt, tag="b_t")
                    nc.vector.tensor_mul(b_t[:], duT[cc][:, c0:c0 + SCW],
                                         ps_bm[:])
                    h_t = scanp.tile([128, SCW], dt, tag="h_t")
                    nc.vector.tensor_tensor_scan(
                        h_t[:], a_t[:], b_t[:], initial=0.0,
                        op0=OP.mult, op1=OP.add)
                    p_t = scanp.tile([128, SCW], mybir.dt.bfloat16, tag="p_t")
                    nc.vector.tensor_mul(p_t[:], h_t[:], ps_cm[:])
                    nc.tensor.matmul(ps_ys[cc][:], identb[:], p_t[:],
                                     start=(d == 0), stop=(d == DS - 1))
            for cc in range(2):
                t1s = scanp.tile([128, SCW], dt, tag="t1s")
                nc.vector.scalar_tensor_tensor(
                    t1s[:], xc2T[cc][:, c0:c0 + SCW],
                    w["D_col"][cc][:], ps_ys[cc][:],
                    op0=OP.mult, op1=OP.add)
                nc.vector.tensor_mul(y2T[cc][:, c0:c0 + SCW], t1s[:],
                                     sluz[cc][:, c0:c0 + SCW])
        if DEBUG:
            for cc in range(2):
                dma(dbg["d_y2T"][cc * 128:(cc + 1) * 128, :], y2T[cc][:])

        # ---- stage H: out_proj ----
        moT = big.tile([DM, LT], dt, tag="sluz0")  # reuse silu_z slot (dead)
        for si in range(NSC):
            c0 = si * SCW
            ps_mo = mm_tile([DM, SCW])
            for cc in range(2):
                nc.tensor.matmul(ps_mo[:],
                                 w["out_proj_wT"][cc][:],
                                 y2T[cc][:, c0:c0 + SCW],
                                 start=(cc == 0), stop=(cc == 1))
            nc.scalar.copy(moT[:, c0:c0 + SCW], ps_mo[:])
        if DEBUG:
            dma(dbg["d_moT"], moT[:])

        # ---- stage I: mlp2 (gelu) + mlp3 + denorm + output ----
        ps_o2 = ps_y.tile([128, RPC], dt, tag="ps_y0")
        ps_o2b = ps_y.tile([2 * PRED - 128, RPC], dt, tag="ps_y1")
        mo_v = moT[:].rearrange("p (b l) -> p b l", b=RPC)
        for pn in range(PN):
            w2t = w2pool.tile([DM, 2 * PRED], dt, tag="w2t")
            dma(w2t[:], IN["w2s"][:, pn:pn + 1, :])
            rhs = mo_v[:, :, GAP + pn:GAP + pn + 1]
            nc.tensor.matmul(ps_o2[:], w2t[:, 0:128], rhs,
                             start=(pn == 0), stop=(pn == PN - 1))
            nc.tensor.matmul(ps_o2b[:], w2t[:, 128:2 * PRED], rhs,
                             start=(pn == 0), stop=(pn == PN - 1))
        o2a = work.tile([128, RPC], dt, tag="o2a")
        nc.scalar.activation(o2a[:], ps_o2[:], AF.Gelu,
                             bias=w["mlp2_b_col"][0][:])
        o2b = work.tile([2 * PRED - 128, RPC], dt, tag="o2b")
        nc.scalar.activation(o2b[:], ps_o2b[:], AF.Gelu,
                             bias=w["mlp2_b_col"][1][:])
        ps_o3 = mm_tile([PRED, RPC])
        nc.tensor.matmul(ps_o3[:], w["mlp3_wT"][0][:], o2a[:],
                         start=True, stop=False)
        nc.tensor.matmul(ps_o3[:], w["mlp3_wT"][1][:], o2b[:],
                         start=False, stop=True)
        o3T = work.tile([PRED, RPC], dt, tag="o3T")
        nc.vector.tensor_copy(o3T[:], ps_o3[:])
        ps_o3t = mm_tile([RPC, PRED])
        nc.tensor.transpose(ps_o3t[:], o3T[:], ident[0:PRED, 0:PRED])

        b3row = work.tile([1, PRED], dt, tag="b3row")
        dma(b3row[:], IN["mlp3_b_row"])
        ps_b3 = mm_tile([RPC, PRED])
        nc.tensor.matmul(ps_b3[:], ones1[:, 0:RPC], b3row[:],
                         start=True, stop=True)
        den = work.tile([RPC, PRED], dt, tag="den")
        nc.vector.tensor_scalar(den[:], ps_b3[:], t_den[:], u_den[:],
                                op0=OP.mult, op1=OP.add)
        out_sb = work.tile([RPC, PRED], dt, tag="out_sb")
        nc.vector.scalar_tensor_tensor(out_sb[:], ps_o3t[:], t_den[:], den[:],
                                       op0=OP.mult, op1=OP.add)
        dma(out_d, out_sb[:])

    nc.compile()
    _cache["nc"] = nc
    return nc


def kernel(**inputs):
    nc = build_program()
    in_maps = prep_inputs(inputs)
    res = run_bass_kernel_spmd(nc, in_maps, list(range(NCORES)))
    return assemble(res.results)


if __name__ == "__main__":
    import reference as R
    inp = R.setup_inputs()
    out = kernel(**{k: np.asarray(v) for k, v in inp.items()})
    print("kernel out", out.shape, out.dtype, np.abs(out).max())
